# revision 16
# baseline (speedup 1.0000x reference)
"""Cross-attention Trainium2 kernel (Bass/Tile), data-parallel over batch.

Problem shapes (hardcoded):
  x       [8, 4096, 1024]  queries input
  context [8, 77, 768]     key/value input
  Wq [1024,1024] Wk [768,1024] Wv [768,1024] Wo [1024,1024] bo [1024]
  out     [8, 4096, 1024]

Sharding: one batch element per NeuronCore (8 cores), weights replicated.
No collectives needed.

Per-core dataflow (all matmuls on PE in float32r):
  xT   = PE-transpose(x chunk)                      [feat, rows]
  qT   = Wq.T @ xT           (lhsT=Wq natural)      [inner, rows]
  kT   = PE-transpose(ctx @ Wk)                     [inner, 77]
  vaug = [v_h | ones(64)] per head                  [77, 128]
  sT_h = kT_h.T @ qT_h       (K=64)                 [77, rows]
  eT_h = exp(sT_h / 8)       (ACT, scale fused)     [77, rows]
  uT_h = vaug_h.T @ eT_h  -> rows 0:64 = attn@v, rows 64:128 = softmax denom
  uN_h = uT_h[0:64] * ACT_recip(uT_h[64:128])       (normalize, no 1-lane ops)
  y    = uN.T @ Wo + bo      (lhsT=uN, rhs=Wo natural; bias added on eviction)
"""

from contextlib import ExitStack

import numpy as np

import concourse.bass as bass
import concourse.tile as tile
from concourse import bacc, mybir
from concourse.bass_utils import run_bass_kernel_spmd
from concourse.masks import make_identity

# ---- shapes -------------------------------------------------------------
B = 8
N = 4096          # query rows per batch element
MC = 77           # context length
QD = 1024         # query feature dim
CD = 768          # context feature dim
INNER = 1024      # H * D
H = 16
D = 64
NCORES = 8

F32 = mybir.dt.float32
F32R = mybir.dt.float32r

CHUNK = 512               # query rows processed per pipeline stage
NCH = N // CHUNK          # 8
RT = CHUNK // 128         # 4 row tiles per chunk
KQ = QD // 128            # 8  k-tiles for q projection
KC = CD // 128            # 6  k-tiles for k/v projections
IT = INNER // 128         # 8  inner-dim tiles
JC = QD // 512            # 2  output column chunks
ATT_SCALE = D ** -0.5     # 1/8, fused into the exp activation


def _r(ap):
    """Reinterpret an fp32 AP as float32r for full-rate PE matmuls."""
    return ap.bitcast(F32R)


def build_bass(repeat=1, dbg=False):
    nc = bacc.Bacc("TRN2", target_bir_lowering=False, debug=False)

    x = nc.dram_tensor("x", [N, QD], F32, kind="ExternalInput").ap()
    ctx = nc.dram_tensor("context", [MC, CD], F32, kind="ExternalInput").ap()
    Wq = nc.dram_tensor("Wq", [QD, INNER], F32, kind="ExternalInput").ap()
    Wk = nc.dram_tensor("Wk", [CD, INNER], F32, kind="ExternalInput").ap()
    Wv = nc.dram_tensor("Wv", [CD, INNER], F32, kind="ExternalInput").ap()
    Wo = nc.dram_tensor("Wo", [INNER, QD], F32, kind="ExternalInput").ap()
    bo = nc.dram_tensor("bo", [QD], F32, kind="ExternalInput").ap()
    y = nc.dram_tensor("y", [N, QD], F32, kind="ExternalOutput").ap()
    if dbg:
        d_bo = nc.dram_tensor("d_bo", [128, QD], F32, kind="ExternalOutput").ap()
        d_xT = nc.dram_tensor("d_xT", [128, KQ, CHUNK], F32, kind="ExternalOutput").ap()
        d_qT = nc.dram_tensor("d_qT", [IT, 128, CHUNK], F32, kind="ExternalOutput").ap()
        d_eT = nc.dram_tensor("d_eT", [H, MC, CHUNK], F32, kind="ExternalOutput").ap()
        d_u = nc.dram_tensor("d_u", [128, IT, CHUNK], F32, kind="ExternalOutput").ap()
        d_kT = nc.dram_tensor("d_kT", [128, IT, MC], F32, kind="ExternalOutput").ap()
        d_va = nc.dram_tensor("d_va", [MC, H, D], F32, kind="ExternalOutput").ap()
        d_ct = nc.dram_tensor("d_ct", [128, KC, MC], F32, kind="ExternalOutput").ap()
        d_pu = nc.dram_tensor("d_pu", [2, 128, CHUNK], F32, kind="ExternalOutput").ap()
        d_den = nc.dram_tensor("d_den", [2, 128, CHUNK], F32, kind="ExternalOutput").ap()
        d_rec = nc.dram_tensor("d_rec", [2, 128, CHUNK], F32, kind="ExternalOutput").ap()

    with tile.TileContext(nc) as tc, ExitStack() as st:
        const = st.enter_context(tc.tile_pool(name="const", bufs=1))
        wpool = st.enter_context(tc.tile_pool(name="wpool", bufs=1))
        wtmp = st.enter_context(tc.tile_pool(name="wtmp", bufs=2))
        xpool = st.enter_context(tc.tile_pool(name="xpool", bufs=2))
        big = st.enter_context(tc.tile_pool(name="big", bufs=2))
        ev = st.enter_context(tc.tile_pool(name="ev", bufs=2))
        ps_tr = st.enter_context(tc.tile_pool(name="ps_tr", bufs=2, space="PSUM"))
        ps_mm = st.enter_context(tc.tile_pool(name="ps_mm", bufs=2, space="PSUM"))
        ps_s = st.enter_context(tc.tile_pool(name="ps_s", bufs=2, space="PSUM"))
        ps_u = st.enter_context(tc.tile_pool(name="ps_u", bufs=2, space="PSUM"))

        iden = const.tile([128, 128], F32)
        make_identity(nc, iden)

        # resident weights: Wq / Wo, [in-feat % 128, in-feat // 128, out].
        # f32r operands must be rounded by their producer, so DMA the raw
        # fp32 tile and round on a vector copy.
        Wq_sb = wpool.tile([128, KQ, INNER], F32R, tag="wq")
        for kt in range(KQ):
            w_raw = wtmp.tile([128, INNER], F32, tag="wraw")
            nc.sync.dma_start(w_raw[:], Wq.rearrange("(ko p) n -> p ko n", p=128)[:, kt, :])
            nc.vector.tensor_copy(Wq_sb[:, kt, :], w_raw[:])
        Wo_sb = wpool.tile([128, IT, QD], F32R, tag="wo")
        for kt in range(IT):
            w_raw = wtmp.tile([128, INNER], F32, tag="wraw")
            nc.sync.dma_start(w_raw[:], Wo.rearrange("(ko p) n -> p ko n", p=128)[:, kt, :])
            nc.vector.tensor_copy(Wo_sb[:, kt, :], w_raw[:])

        # bias broadcast to all partitions
        bo_bc = const.tile([128, QD], F32)
        nc.sync.dma_start(bo_bc[:], bo[None, :].to_broadcast((128, QD)))

        # context, natural then transposed
        ctx_sb = const.tile([MC, CD], F32)
        nc.sync.dma_start(ctx_sb[:], ctx)
        ctxT = const.tile([128, KC, MC], F32R)
        for ft in range(KC):
            pt = ps_tr.tile([128, 128], F32, tag="tr")
            nc.tensor.transpose(
                pt[:, :MC], ctx_sb[:, ft * 128 : (ft + 1) * 128], iden[:MC, :MC]
            )
            nc.vector.tensor_copy(ctxT[:, ft, :], pt[:, :MC])

        # k and v natural [77, 1024], PSUM-accumulated over feature k-tiles
        k_nat = const.tile([MC, INNER], F32, tag="knat")
        # reuse the attention-phase PSUM tags so each pool stays at 2 banks
        v_ps = [ps_s.tile([MC, 512], F32, tag="s", name=f"vps{j}") for j in range(2)]
        k_ps = [ps_u.tile([MC, 512], F32, tag="u", name=f"kps{j}") for j in range(2)]
        for kt in range(KC):
            wk_raw = wtmp.tile([128, INNER], F32, tag="wraw")
            nc.sync.dma_start(wk_raw[:], Wk.rearrange("(ko p) n -> p ko n", p=128)[:, kt, :])
            wk_t = wtmp.tile([128, INNER], F32R, tag="wkv")
            nc.vector.tensor_copy(wk_t[:], wk_raw[:])
            wv_raw = wtmp.tile([128, INNER], F32, tag="wraw")
            nc.sync.dma_start(wv_raw[:], Wv.rearrange("(ko p) n -> p ko n", p=128)[:, kt, :])
            wv_t = wtmp.tile([128, INNER], F32R, tag="wkv")
            nc.vector.tensor_copy(wv_t[:], wv_raw[:])
            for j in range(2):
                nc.tensor.matmul(
                    k_ps[j][:],
                    ctxT[:, kt, :],
                    wk_t[:, j * 512 : (j + 1) * 512],
                    start=(kt == 0),
                    stop=(kt == KC - 1),
                )
                nc.tensor.matmul(
                    v_ps[j][:],
                    ctxT[:, kt, :],
                    wv_t[:, j * 512 : (j + 1) * 512],
                    start=(kt == 0),
                    stop=(kt == KC - 1),
                )

        # kT [128, 8, 77] via PE transpose of k_nat
        kT = const.tile([128, IT, MC], F32R, tag="kT")
        for j in range(2):
            nc.vector.tensor_copy(k_nat[:, j * 512 : (j + 1) * 512], k_ps[j][:])
        for it in range(IT):
            pt = ps_tr.tile([128, 128], F32, tag="tr")
            nc.tensor.transpose(
                pt[:, :MC], k_nat[:, it * 128 : (it + 1) * 128], iden[:MC, :MC]
            )
            nc.vector.tensor_copy(kT[:, it, :], pt[:, :MC])

        # vaug [77, h, 128] = [v_h | v_h]: one matmul per head yields attn@v
        # on BOTH partition halves; allones yields the denominator on both.
        # Each head then reads the half matching its u_sb slot, so no f32r
        # matmul ever needs a PSUM partition offset (ISA-rejected) and all
        # DVE ops stay lane-aligned.
        ones_f32 = const.tile([MC, 128], F32)
        nc.gpsimd.memset(ones_f32[:], 1.0)
        allones = const.tile([MC, 128], F32R)
        nc.vector.tensor_copy(allones[:], ones_f32[:])
        vaug = const.tile([MC, H, 128], F32R, tag="vaug")
        for h in range(H):
            j, off = divmod(h * D, 512)
            nc.vector.tensor_copy(vaug[:, h, :D], v_ps[j][:, off : off + D])
            nc.vector.tensor_copy(vaug[:, h, D:], v_ps[j][:, off : off + D])
        if dbg:
            nc.sync.dma_start(d_bo[:], bo_bc[:])
            nc.sync.dma_start(d_kT[:], kT[:].bitcast(F32))
            nc.sync.dma_start(d_va[:], vaug[:, :, :D].bitcast(F32))
            nc.sync.dma_start(d_ct[:], ctxT[:].bitcast(F32))

        # ---- main loop over query-row chunks ----------------------------
        # repeat>1 re-runs the whole loop writing identical output; used only
        # for slope-based wall-clock timing (amortizes host/RPC overhead).
        for c in [ci for _ in range(repeat) for ci in range(NCH)]:
            r0 = c * CHUNK

            # load + transpose x chunk -> xT [128, KQ, CHUNK]
            xT = big.tile([128, KQ, CHUNK], F32R, tag="xT")
            for rt in range(RT):
                x_nat = xpool.tile([128, QD], F32, tag="xnat")
                nc.sync.dma_start(x_nat[:], x[r0 + rt * 128 : r0 + (rt + 1) * 128, :])
                for ft in range(KQ):
                    pt = ps_tr.tile([128, 128], F32, tag="tr")
                    nc.tensor.transpose(
                        pt[:], x_nat[:, ft * 128 : (ft + 1) * 128], iden[:]
                    )
                    nc.vector.tensor_copy(
                        xT[:, ft, rt * 128 : (rt + 1) * 128], pt[:]
                    )

            # u_sb accumulates normalized per-head outputs, transposed layout
            u_sb = big.tile([128, IT, CHUNK], F32R, tag="u")

            for it in range(IT):
                # qT for this inner tile: [128, CHUNK]
                pq = ps_mm.tile([128, 512], F32, tag="mm")
                for kt in range(KQ):
                    nc.tensor.matmul(
                        pq[:],
                        Wq_sb[:, kt, it * 128 : (it + 1) * 128],
                        xT[:, kt, :],
                        start=(kt == 0),
                        stop=(kt == KQ - 1),
                    )
                qT_it = ev.tile([128, CHUNK], F32R, tag="qT")
                nc.vector.tensor_copy(qT_it[:], pq[:])
                if dbg and c == 0:
                    nc.sync.dma_start(d_qT[it], qT_it[:].bitcast(F32))

                for hh in range(2):  # heads 2*it and 2*it+1
                    h = 2 * it + hh
                    po = hh * D
                    # scoresT [77, CHUNK] = kT_h.T @ qT_h  (K = 64)
                    ps = ps_s.tile([MC, 512], F32, tag="s")
                    nc.tensor.matmul(
                        ps[:],
                        kT[po : po + D, it, :],
                        qT_it[po : po + D, :],
                        start=True,
                        stop=True,
                    )
                    # expT = exp(scoresT / 8)
                    eT = ev.tile([MC, CHUNK], F32R, tag="eT")
                    nc.scalar.activation(
                        eT[:], ps[:], mybir.ActivationFunctionType.Exp,
                        scale=ATT_SCALE,
                    )
                    if dbg and c == 0:
                        nc.sync.dma_start(d_eT[h], eT[:].bitcast(F32))
                    # attn@v on both halves; denominator on both halves
                    pu = ps_u.tile([128, 512], F32, tag="u")
                    nc.tensor.matmul(
                        pu[:], vaug[:, h, :], eT[:], start=True, stop=True
                    )
                    den = ps_u.tile([128, 512], F32, tag="u", name="den")
                    nc.tensor.matmul(
                        den[:], allones[:], eT[:], start=True, stop=True
                    )
                    # full-width base-0 recip (den halves are identical);
                    # the custom DVE op mishandles partition offsets
                    rec = ev.tile([128, CHUNK], F32, tag="rec")
                    nc.vector.reciprocal_approx_fast(rec[:], den[:])
                    if dbg and c == 0 and it == 0:
                        dtmp1 = ev.tile([128, CHUNK], F32, tag="y", name="dtmp1")
                        nc.vector.tensor_copy(dtmp1[:], pu[:])
                        nc.sync.dma_start(d_pu[hh], dtmp1[:])
                        dtmp2 = ev.tile([128, CHUNK], F32, tag="y", name="dtmp2")
                        nc.vector.tensor_copy(dtmp2[:], den[:])
                        nc.sync.dma_start(d_den[hh], dtmp2[:])
                        nc.sync.dma_start(d_rec[hh], rec[:])
                    nc.vector.tensor_mul(
                        u_sb[po : po + D, it, :],
                        pu[po : po + D, :],
                        rec[po : po + D, :],
                    )

            if dbg and c == 0:
                nc.sync.dma_start(d_xT[:], xT[:].bitcast(F32))
                nc.sync.dma_start(d_u[:], u_sb[:].bitcast(F32))

            # y = u.T @ Wo + bo, written back per 128-row x 512-col tile
            for rt in range(RT):
                for jc in range(JC):
                    py = ps_mm.tile([128, 512], F32, tag="mm")
                    for kt in range(IT):
                        nc.tensor.matmul(
                            py[:],
                            u_sb[:, kt, rt * 128 : (rt + 1) * 128],
                            Wo_sb[:, kt, jc * 512 : (jc + 1) * 512],
                            start=(kt == 0),
                            stop=(kt == IT - 1),
                        )
                    y_sb = ev.tile([128, 512], F32, tag="y")
                    nc.vector.tensor_add(
                        y_sb[:], py[:], bo_bc[:, jc * 512 : (jc + 1) * 512]
                    )
                    nc.sync.dma_start(
                        y[r0 + rt * 128 : r0 + (rt + 1) * 128,
                          jc * 512 : (jc + 1) * 512],
                        y_sb[:],
                    )

    nc.compile()
    return nc


_NC = None


def _get_nc():
    global _NC
    if _NC is None:
        _NC = build_bass()
    return _NC


def _run(inputs, trace=False):
    nc = _get_nc()
    in_maps = []
    for b in range(B):
        in_maps.append(
            {
                "x": np.ascontiguousarray(np.asarray(inputs["x"])[b], dtype=np.float32),
                "context": np.ascontiguousarray(
                    np.asarray(inputs["context"])[b], dtype=np.float32
                ),
                "Wq": np.ascontiguousarray(np.asarray(inputs["Wq"]), dtype=np.float32),
                "Wk": np.ascontiguousarray(np.asarray(inputs["Wk"]), dtype=np.float32),
                "Wv": np.ascontiguousarray(np.asarray(inputs["Wv"]), dtype=np.float32),
                "Wo": np.ascontiguousarray(np.asarray(inputs["Wo"]), dtype=np.float32),
                "bo": np.ascontiguousarray(np.asarray(inputs["bo"]), dtype=np.float32),
            }
        )
    res = run_bass_kernel_spmd(nc, in_maps, core_ids=list(range(NCORES)), trace=trace)
    out = np.stack([res.results[c]["y"] for c in range(NCORES)], axis=0)
    return out.astype(np.float32), res


def run_traced(inputs):
    out, res = _run(inputs, trace=True)
    return out, res


def kernel(x, context, Wq, Wk, Wv, Wo, bo):
    out, _ = _run(
        {"x": x, "context": context, "Wq": Wq, "Wk": Wk, "Wv": Wv, "Wo": Wo, "bo": bo}
    )
    return out
